# Initial kernel scaffold
#
"""BLOutputLayer forward: out[i] = features[rules[i]] — a rule-book gather.

Strategy (8 NeuronCores, data-parallel over output rows):
  - rules [524288] int -> 8 shards of 65536 rows, one per core.
  - features [200000, 64] f32 (51.2 MB) replicated to every core's DRAM.
  - Per core: loop over tiles of 128x128 indices; load indices to SBUF,
    gpsimd indirect DMA gather (one 256 B descriptor per row) from the
    DRAM feature table into an SBUF tile [128, 128*64] f32, then store
    the tile contiguously to the output DRAM buffer.
  - Host concatenates the 8 per-core outputs.
"""

import numpy as np

import concourse.bacc as bacc
import concourse.mybir as mybir
import concourse.tile as tile
from concourse.bass import IndirectOffsetOnAxis
from concourse.bass_utils import run_bass_kernel_spmd

N_ACTIVE = 200000
C = 64
N_ROWS = 524288
N_CORES = 8
ROWS_PER_CORE = N_ROWS // N_CORES  # 65536
P = 128  # SBUF partitions
K = 128  # indices per partition per tile
TILE_ROWS = P * K  # 16384
N_TILES = ROWS_PER_CORE // TILE_ROWS  # 4

_cache = {}


def _build():
    nc = bacc.Bacc("TRN2", target_bir_lowering=False)
    features = nc.dram_tensor(
        "features", [N_ACTIVE, C], mybir.dt.float32, kind="ExternalInput"
    )
    rules = nc.dram_tensor(
        "rules", [N_TILES, P, K], mybir.dt.int32, kind="ExternalInput"
    )
    out = nc.dram_tensor(
        "out", [N_TILES, P, K * C], mybir.dt.float32, kind="ExternalOutput"
    )

    with tile.TileContext(nc) as tc:
        with (
            tc.tile_pool(name="idx", bufs=N_TILES) as idx_pool,
            tc.tile_pool(name="data", bufs=3) as data_pool,
        ):
            for t in range(N_TILES):
                idx_tile = idx_pool.tile([P, K], mybir.dt.int32, tag="idx")
                data_tile = data_pool.tile([P, K * C], mybir.dt.float32, tag="data")
                nc.sync.dma_start(out=idx_tile[:], in_=rules[t])
                nc.gpsimd.indirect_dma_start(
                    out=data_tile[:],
                    out_offset=None,
                    in_=features[:],
                    in_offset=IndirectOffsetOnAxis(ap=idx_tile[:], axis=0),
                )
                nc.sync.dma_start(out=out[t], in_=data_tile[:])
    return nc


def run(features, rules, trace=False):
    features = np.ascontiguousarray(np.asarray(features), dtype=np.float32)
    rules_i32 = np.ascontiguousarray(np.asarray(rules)).astype(np.int32)
    rules_i32 = rules_i32.reshape(N_CORES, N_TILES, P, K)

    if "nc" not in _cache:
        _cache["nc"] = _build()
    nc = _cache["nc"]

    in_maps = [{"features": features, "rules": rules_i32[c]} for c in range(N_CORES)]
    res = run_bass_kernel_spmd(nc, in_maps, list(range(N_CORES)), trace=trace)
    outs = [res.results[c]["out"].reshape(ROWS_PER_CORE, C) for c in range(N_CORES)]
    full = np.concatenate(outs, axis=0)
    return full, res


def kernel(**inputs):
    full, _ = run(inputs["features"], inputs["rules"])
    return full


# revision 4
# speedup vs baseline: 1.4821x; 1.4821x over previous
"""BLOutputLayer forward: out[i] = features[rules[i]] — a rule-book gather.

Strategy (8 NeuronCores, data-parallel over output rows):
  - rules [524288] int -> 8 shards of 65536 rows, one per core.
  - features [200000, 64] f32 (51.2 MB) replicated to every core's DRAM.
  - Per core: 512 indirect DMA gathers of 128 rows each (one index per
    SBUF partition — the only offset layout the runtime DGE supports),
    grouped G at a time into an SBUF tile that is stored contiguously.
  - Index layout is pre-arranged on host so that gather g's indices sit
    at idx_tile[:, g], and the store order reproduces the shard order.
"""

import numpy as np

import concourse.bacc as bacc
import concourse.mybir as mybir
import concourse.tile as tile
from concourse.bass import IndirectOffsetOnAxis
from concourse.bass_utils import run_bass_kernel_spmd

N_ACTIVE = 200000
C = 64
N_ROWS = 524288
N_CORES = 8
ROWS_PER_CORE = N_ROWS // N_CORES  # 65536
P = 128  # SBUF partitions
N_GATHERS = ROWS_PER_CORE // P  # 512 indirect DMAs per core
G = 32  # gathers per store group
N_GROUPS = N_GATHERS // G  # 16

_cache = {}


def _build(reps=1):
    nc = bacc.Bacc("TRN2", target_bir_lowering=False)
    features = nc.dram_tensor(
        "features", [N_ACTIVE, C], mybir.dt.float32, kind="ExternalInput"
    )
    # host pre-arranges indices: rules_w[p, i] = shard[i * P + p]
    rules = nc.dram_tensor(
        "rules", [P, N_GATHERS], mybir.dt.int32, kind="ExternalInput"
    )
    # out[grp, p, g, :] = row grp*G*P + ... stored so host reshape works
    out = nc.dram_tensor(
        "out", [N_GROUPS, P, G, C], mybir.dt.float32, kind="ExternalOutput"
    )

    with tile.TileContext(nc) as tc:
        with (
            tc.tile_pool(name="idx", bufs=1) as idx_pool,
            tc.tile_pool(name="data", bufs=3) as data_pool,
        ):
            idx_tile = idx_pool.tile([P, N_GATHERS], mybir.dt.int32, tag="idx")
            nc.sync.dma_start(out=idx_tile[:], in_=rules[:])
            for _rep in range(reps):
                for grp in range(N_GROUPS):
                    data_tile = data_pool.tile([P, G, C], mybir.dt.float32, tag="data")
                    for g in range(G):
                        j = grp * G + g
                        nc.gpsimd.indirect_dma_start(
                            out=data_tile[:, g],
                            out_offset=None,
                            in_=features[:],
                            in_offset=IndirectOffsetOnAxis(
                                ap=idx_tile[:, j : j + 1], axis=0
                            ),
                        )
                    nc.sync.dma_start(out=out[grp], in_=data_tile[:])
    nc.finalize()
    return nc


def run(features, rules, trace=False):
    features = np.ascontiguousarray(np.asarray(features), dtype=np.float32)
    rules_i32 = np.ascontiguousarray(np.asarray(rules)).astype(np.int32)
    # per core: rules_w[p, i] = shard[i * P + p]
    shards = rules_i32.reshape(N_CORES, N_GATHERS, P)
    rules_w = shards.transpose(0, 2, 1).copy()  # [core, P, N_GATHERS]

    if "nc" not in _cache:
        _cache["nc"] = _build()
    nc = _cache["nc"]

    in_maps = [{"features": features, "rules": rules_w[c]} for c in range(N_CORES)]
    res = run_bass_kernel_spmd(nc, in_maps, list(range(N_CORES)), trace=trace)
    # out[grp, p, g, :] holds shard row grp*G*P + ??? -> need inverse of the
    # rules_w layout: gather j=grp*G+g, partition p holds shard row j*P+p.
    outs = []
    for c in range(N_CORES):
        o = res.results[c]["out"]  # [N_GROUPS, P, G, C]
        o = o.transpose(0, 2, 1, 3).reshape(ROWS_PER_CORE, C)  # [grp, g, p] order
        outs.append(o)
    full = np.concatenate(outs, axis=0)
    return full, res


def kernel(**inputs):
    full, _ = run(inputs["features"], inputs["rules"])
    return full
